# revision 21
# baseline (speedup 1.0000x reference)
"""CrossAttention Trainium2 kernel.

B=4, T=2048, HIN=1024, H=16 heads, E=64. 8 NeuronCores, tensor-parallel
over heads (2 heads per core). Each core receives the full (transposed,
bf16-cast) activations plus its 128-column slice of Wq/Wk/Wv, computes
q/k/v projections and full softmax attention for its 2 heads, and writes
its output slice in [e, t] layout (host reassembles).
"""

import numpy as np
import ml_dtypes

import concourse.bass as bass
import concourse.mybir as mybir
import concourse.tile as tile
from concourse import bacc
from concourse.bass_utils import run_bass_kernel_spmd
from concourse.tile import TileContext, ScopedClock

BF16 = mybir.dt.bfloat16
F32 = mybir.dt.float32

B, T, HIN, H, E = 4, 2048, 1024, 16, 64
NCORES = 8
HL = H // NCORES          # heads per core = 2
ES = HL * E               # 128 (e-shard width)
BT = B * T                # 8192
KI = HIN // 128           # 8 contraction k-tiles for projections
NBLK = BT // 128          # 64 t-blocks of 128
SCALE = float(E) ** -0.25

_EXP = mybir.ActivationFunctionType.Exp


def _patch_tail_drain():
    """walrus in this container allows only ONE sync-wait per instruction;
    Tile's kernel-tail drain accumulates one wait per live proc. Spread the
    waits across single-wait NOPs."""
    if getattr(TileContext, "_tail_drain_patched", False):
        return

    def _drain_and_barrier(self, tick_clock, wait_clock):
        probe = self.nc.sync.nop(nofuse=True, hint="tail_wait_probe")
        wait_clock.add_sem_waits(
            probe.ins, ScopedClock({None: tick_clock.global_clock})
        )
        si = probe.ins.sync_info
        waits = list(si.on_wait) if si is not None else []
        if len(waits) > 1:
            probe.ins.sync_info = mybir.SyncInfo(
                on_wait=waits[:1], on_update=list(si.on_update)
            )
            for i in range(1, len(waits)):
                n2 = self.nc.sync.nop(nofuse=True, hint=f"tail_wait_{i}")
                n2.ins.sync_info = mybir.SyncInfo(on_wait=[waits[i]], on_update=[])
        self.nc.sync.drain()
        self.nc.all_engine_barrier()
        popped = self.nc._tile_sem_poison_stack.pop()
        assert popped is self._sem_poison
        self.nc.clear_and_free_semaphores(list(self.sems.allocated().values()))
        self.nc.all_engine_barrier()

    TileContext._drain_and_barrier = _drain_and_barrier
    TileContext._tail_drain_patched = True


def emit_body(nc, tc, ctx_pools, xq, xkv, wq, wk, wv, out, dbg=None, phase1_only=False):
    """Emit one full forward pass."""
    with tc.tile_pool(name="persist", bufs=1) as persist:
        # persistent SBUF tensors for the attention phase
        qT_s = persist.tile([128, BT], BF16)          # [e_shard, b*t]
        kT_s = persist.tile([128, BT], BF16)          # [e_shard, b*t]
        # v in natural layout + interleaved ones column per head:
        # per 128-t-block: [h0 e(64) | 1 | h1 e(64) | 1] -> 130 cols
        v_sb = persist.tile([128, NBLK, 2 * (E + 1)], BF16)
        nc.vector.memset(v_sb, 1.0)

        # ---------------- Phase 1: projections ----------------
        with tc.tile_pool(name="xin", bufs=2) as xin, \
             tc.tile_pool(name="win", bufs=1) as win, \
             tc.tile_pool(name="ps_q", bufs=2, space="PSUM") as ps_q, \
             tc.tile_pool(name="ps_k", bufs=2, space="PSUM") as ps_k, \
             tc.tile_pool(name="ps_v", bufs=2, space="PSUM") as ps_v:
            wq_sb = win.tile([128, KI, ES], BF16, tag="wq")
            wk_sb = win.tile([128, KI, ES], BF16, tag="wk")
            wv_sb = win.tile([128, KI, ES], BF16, tag="wv")
            nc.sync.dma_start(out=wq_sb, in_=wq[:, :, :])
            nc.sync.dma_start(out=wk_sb, in_=wk[:, :, :])
            nc.sync.dma_start(out=wv_sb, in_=wv[:, :, :])

            NCH = BT // 512  # 16 chunks of 512 t-columns
            for nb in range(NCH):
                cs = slice(nb * 512, (nb + 1) * 512)
                xq_sb = xin.tile([128, KI, 512], BF16, tag="xq")
                xkv_sb = xin.tile([128, KI, 512], BF16, tag="xkv")
                nc.sync.dma_start(out=xq_sb, in_=xq[:, :, cs])
                nc.sync.dma_start(out=xkv_sb, in_=xkv[:, :, cs])

                # qT[e, t] / kT[e, t]: lhsT = W[i-block, e], rhs = xT[i-block, t]
                psq = ps_q.tile([128, 512], F32)
                psk = ps_k.tile([128, 512], F32)
                for i in range(KI):
                    nc.tensor.matmul(psq, wq_sb[:, i, :], xq_sb[:, i, :],
                                     start=(i == 0), stop=(i == KI - 1))
                for i in range(KI):
                    nc.tensor.matmul(psk, wk_sb[:, i, :], xkv_sb[:, i, :],
                                     start=(i == 0), stop=(i == KI - 1))
                nc.vector.tensor_copy(qT_s[:, cs], psq)
                nc.vector.tensor_copy(kT_s[:, cs], psk)

                # v[t, e] natural: lhsT = xkv[i-block, t-subblock], rhs = W[i-block, e]
                psv = ps_v.tile([128, 4, ES], F32)
                for sub in range(4):
                    for i in range(KI):
                        nc.tensor.matmul(
                            psv[:, sub, :],
                            xkv_sb[:, i, sub * 128:(sub + 1) * 128],
                            wv_sb[:, i, :],
                            start=(i == 0), stop=(i == KI - 1))
                # scatter into v_sb blocks (skip the ones columns)
                blk0 = nb * 4
                src = psv.rearrange("p s (h e) -> p s h e", h=HL)
                dst = v_sb[:, blk0:blk0 + 4, :].rearrange(
                    "p s (h e1) -> p s h e1", h=HL)[:, :, :, :E]
                nc.vector.tensor_copy(dst, src)

        if dbg is not None:
            dbg_q, dbg_k, dbg_v, dbg_p, dbg_o = dbg[:5]
            nc.sync.dma_start(out=dbg_q[:, :], in_=qT_s)
            nc.sync.dma_start(out=dbg_k[:, :], in_=kT_s)
            nc.sync.dma_start(
                out=dbg_v[:, :], in_=v_sb.rearrange("p a b -> p (a b)"))

        if phase1_only:
            # dump something so outputs are written
            with tc.tile_pool(name="po1", bufs=1) as po1:
                t = po1.tile([128, 512], F32)
                nc.vector.tensor_copy(t, qT_s[:, 0:512])
                nc.sync.dma_start(out=out[:, 0:512], in_=t)
            return

        # ---------------- Phase 2: attention ----------------
        QC = 1024                 # q-chunk width
        NQC = T // QC             # 2 chunks per batch
        KT = T // 128             # 16 k-tiles per batch
        with tc.tile_pool(name="pP", bufs=6) as pP, \
             tc.tile_pool(name="pp_x", bufs=1, space="PSUM") as pp_x, \
             tc.tile_pool(name="pp_o", bufs=2, space="PSUM") as pp_o, \
             tc.tile_pool(name="dn", bufs=2) as dn, \
             tc.tile_pool(name="dr", bufs=2, space="DRAM") as dr, \
             tc.tile_pool(name="po", bufs=2) as po:
            for b in range(B):
                for qc in range(NQC):
                    q0 = b * T + qc * QC  # column base into qT_s
                    X = pp_x.tile([128, 2, QC], F32)       # S^T staging (4 banks)
                    oT = [pp_o.tile([E + 1, QC], F32, tag="ot", name=f"oT{h}")
                          for h in range(HL)]
                    P_tiles = [[None] * HL for _ in range(KT)]
                    for kt in range(KT):
                        k0 = b * T + kt * 128
                        blk = b * KT + kt
                        for h in range(HL):
                            s = h  # X half
                            hp = slice(h * E, (h + 1) * E)
                            # S^T[k_tile, q_chunk] = K^T.T @ Q^T (contraction e=64,
                            # two heads row-packed onto PE rows 0:64 / 64:128)
                            for ns in range(QC // 512):
                                nc.tensor.matmul(
                                    X[:, s, ns * 512:(ns + 1) * 512],
                                    kT_s[hp, k0:k0 + 128],
                                    qT_s[hp, q0 + ns * 512: q0 + (ns + 1) * 512],
                                    start=True, stop=True)
                            # P = exp(S^T) -> bf16
                            Pt = pP.tile([128, QC], BF16, tag="P")
                            P_tiles[kt][h] = Pt
                            nc.scalar.activation(out=Pt, in_=X[:, s, :], func=_EXP)
                            if dbg is not None and b == 0 and qc == 0 \
                                    and kt == 0 and h == 0:
                                nc.sync.dma_start(out=dbg_p[:, :], in_=Pt)
                        for h in range(HL):
                            # oT[e|den, q] += [v|1].T @ P  (contraction k=128)
                            vp = v_sb[:, blk, h * (E + 1): (h + 1) * (E + 1)]
                            Pt = P_tiles[kt][h]
                            for ns in range(QC // 512):
                                nc.tensor.matmul(
                                    oT[h][:, ns * 512:(ns + 1) * 512],
                                    vp,
                                    Pt[:, ns * 512:(ns + 1) * 512],
                                    start=(kt == 0), stop=(kt == KT - 1))
                    # ---- normalize: out = oT[0:E] / oT[E] ----
                    if dbg is not None and b == 0 and qc == 0:
                        ocp = dn.tile([E + 1, QC], F32, tag="ocp")
                        nc.vector.tensor_copy(ocp, oT[0])
                        nc.sync.dma_start(out=dbg_o[:, :], in_=ocp)
                    # r = 1/denominator, computed at partition 64 (no partition
                    # shift), bounced through DRAM for the partition-broadcast
                    rscr = dr.tile([HL, QC], F32, tag="rscr")
                    for h in range(HL):
                        dcp = dn.tile([E + 1, QC], F32, tag="d", name=f"dcp{h}")
                        nc.vector.tensor_copy(dcp[E:E + 1, :], oT[h][E:E + 1, :])
                        nc.sync.dma_start(out=rscr[h, :], in_=dcp[E:E + 1, :])
                    for h in range(HL):
                        drep = dn.tile([E, QC], F32, tag="drep", name=f"drep{h}")
                        nc.sync.dma_start(
                            out=drep, in_=rscr[h, :].partition_broadcast(E))
                        rrep = dn.tile([E, QC], F32, tag="rrep", name=f"rrep{h}")
                        nc.vector.reciprocal_approx_fast(out=rrep, in_=drep)
                        osb = po.tile([E, QC], F32, tag="osb")
                        nc.vector.tensor_mul(osb, oT[h][:E, :], rrep)
                        nc.sync.dma_start(
                            out=out[h * E:(h + 1) * E, q0:q0 + QC], in_=osb)
                        if dbg is not None and b == 0 and qc == 0 and h == 0:
                            nc.sync.dma_start(out=dbg[5][:, :], in_=rrep)


def build_nc(reps: int = 1, phase1_only=False):
    _patch_tail_drain()
    nc = bacc.Bacc(None)
    xq = nc.declare_dram_parameter("xq", [128, KI, BT], BF16, isOutput=False)
    xkv = nc.declare_dram_parameter("xkv", [128, KI, BT], BF16, isOutput=False)
    wq = nc.declare_dram_parameter("wq", [128, KI, ES], BF16, isOutput=False)
    wk = nc.declare_dram_parameter("wk", [128, KI, ES], BF16, isOutput=False)
    wv = nc.declare_dram_parameter("wv", [128, KI, ES], BF16, isOutput=False)
    out = nc.declare_dram_parameter("out", [ES, BT], F32, isOutput=True)
    with TileContext(nc) as tc:
        for _ in range(reps):
            emit_body(nc, tc, None, xq, xkv, wq, wk, wv, out, phase1_only=phase1_only)
    nc.finalize()
    return nc


def make_in_maps(query, key_value, Wq, Wk, Wv):
    """Host-side sharding/layout. Returns per-core input maps."""
    bf = ml_dtypes.bfloat16
    xq = np.ascontiguousarray(query.reshape(BT, HIN).T.reshape(
        KI, 128, BT).transpose(1, 0, 2)).astype(bf)
    xkv = np.ascontiguousarray(key_value.reshape(BT, HIN).T.reshape(
        KI, 128, BT).transpose(1, 0, 2)).astype(bf)
    wq_s = (Wq.astype(np.float32) * SCALE).astype(bf)
    wk_s = (Wk.astype(np.float32) * SCALE).astype(bf)
    wv_s = Wv.astype(bf)
    in_maps = []
    for c in range(NCORES):
        cols = slice(c * ES, (c + 1) * ES)
        in_maps.append({
            "xq": xq,
            "xkv": xkv,
            "wq": np.ascontiguousarray(wq_s[:, cols].reshape(KI, 128, ES).transpose(1, 0, 2)),
            "wk": np.ascontiguousarray(wk_s[:, cols].reshape(KI, 128, ES).transpose(1, 0, 2)),
            "wv": np.ascontiguousarray(wv_s[:, cols].reshape(KI, 128, ES).transpose(1, 0, 2)),
        })
    return in_maps


def assemble_output(results):
    full = np.concatenate(
        [np.asarray(results[c]["out"]) for c in range(NCORES)], axis=0)
    return np.ascontiguousarray(full.T).reshape(B, T, H * E).astype(np.float32)


_NC_CACHE = {}


def kernel(query, key_value, Wq, Wk, Wv):
    query = np.asarray(query, dtype=np.float32)
    key_value = np.asarray(key_value, dtype=np.float32)
    Wq = np.asarray(Wq, dtype=np.float32)
    Wk = np.asarray(Wk, dtype=np.float32)
    Wv = np.asarray(Wv, dtype=np.float32)

    if "nc" not in _NC_CACHE:
        _NC_CACHE["nc"] = build_nc(reps=1)
    nc = _NC_CACHE["nc"]
    in_maps = make_in_maps(query, key_value, Wq, Wk, Wv)
    res = run_bass_kernel_spmd(nc, in_maps, list(range(NCORES)))
    return assemble_output(res.results)


if __name__ == "__main__":
    rng = np.random.default_rng(0)
    q = rng.standard_normal((B, T, HIN), dtype=np.float32)
    kv = rng.standard_normal((B, T, HIN), dtype=np.float32)
    s = 1.0 / np.sqrt(HIN)
    wq = rng.uniform(-s, s, (HIN, H * E)).astype(np.float32)
    wk = rng.uniform(-s, s, (HIN, H * E)).astype(np.float32)
    wv = rng.uniform(-s, s, (HIN, H * E)).astype(np.float32)
    out = kernel(query=q, key_value=kv, Wq=wq, Wk=wk, Wv=wv)
    print("out", out.shape, out.dtype, np.abs(out).mean())
